# revision 30
# baseline (speedup 1.0000x reference)
"""GAU denoising transformer forward pass on 8 Trainium2 NeuronCores.

Strategy: data-parallel over batch (B=16 -> 2 images per core). Each core
runs an identical NEFF on its own pair of images with all weights
replicated. Per core the residual stream is kept transposed in SBUF
(hTp: 3 pair tiles of [128, 2, 512]; H on partitions x 512 tokens =
2 images x 256 patches) in fp32 for the whole 24-layer stack.

The three large uvqk GEMM groups per layer (u/q/k weight-stationary, v
activation-stationary) and attn@v run in fp8 e4m3 with
perf_mode=DoubleRow: operands are laid out as [128, 2, F] k-pair APs so
each matmul instruction contracts 256 (2x the bf16 rate). The
out-projection stays bf16: its error feeds the residual directly and
dominates the fp8 error budget (measured). Scales:
  - wuv quantized at x64 (keeps 0.02-scale weights out of the fp8
    subnormal range); the 1/64 is folded into the rmsnorm reciprocal
    (rb/rt) applied at psum evacuation.
  - attention probabilities are scaled x16 into fp8 normal range; the
    1/16 is folded into the residual-add scale.
  - activations (h, v, attn) are quantized 1:1.

Scheduling (the important part -- engines pop their queues in order, so
emission order IS the schedule):
  - PE: qk+swap -> v(tk0,tk1) -> sumsq/scatter -> v(tk2,tk3) -> gather
    -> u -> scores -> transposes -> attn@v -> out-proj. The out-proj is
    contraction-outer (all 6 dps banks accumulate in parallel, each
    e-round firing as its og chunk is gated) with the last round
    per-hp-chained into the residual/square/fp8-copy tail so psum banks
    and h state free incrementally for the next layer's front.
  - RoPE runs entirely in bf16 (2x DVE) off SBUF copies of the swap
    matmuls; attn@v evacuations are batched to one [128,512] bank per
    e-chunk (both images); squares and fp8 copies are batched per
    h-pair ([128,1024] ops).
"""

import sys

for _p in ("/opt/trn_rl_repo",):
    if _p not in sys.path:
        sys.path.append(_p)

import numpy as np
import ml_dtypes

BF = ml_dtypes.bfloat16
F8 = ml_dtypes.float8_e4m3

IMG = 128
P = 8
H = 768
E = 1536
KD = 128          # key size
L = 256           # patches per image
PD = 192          # patch dim
NL = 24
B = 16
NCORES = 8
TOK = 512         # tokens per core (2 images x 256)
HC = H // 128     # 6 h-chunks
HP = HC // 2      # 3 h-chunk pairs
EC = E // 128     # 12 e-chunks
EP = EC // 2      # 6 e-chunk pairs
WUV_W = E + 2 * 128 + E   # permuted wuv width: u | q | k | v
V0 = E + 2 * 128          # column offset of v block

WSCALE = 64.0     # fp8 weight quantization scale
ASCALE = 16.0     # attn-probability fp8 scale
OP_FP8 = False    # out-projection GEMM in fp8 DoubleRow (vs bf16)
# out-proj psum -> residual scale: og carries x16 (attn), wout x64 if fp8
RES_SCALE = 1.0 / (ASCALE * (WSCALE if OP_FP8 else 1.0))


def _build(nl=NL, repeat=1):
    """Build + compile the Bass module. Returns nc."""
    import concourse.tile as tile
    from concourse import bacc, mybir
    from concourse.masks import make_identity

    F32 = mybir.dt.float32
    BF16 = mybir.dt.bfloat16
    FP8 = mybir.dt.float8e4
    AF = mybir.ActivationFunctionType
    DR = mybir.MatmulPerfMode.DoubleRow
    MUL = mybir.AluOpType.mult
    ADD = mybir.AluOpType.add

    nc = bacc.Bacc("TRN2", target_bir_lowering=False, debug=False,
                   num_devices=NCORES)

    d_xpt = nc.dram_tensor("xpt", [128, 2, TOK], BF16, kind="ExternalInput")
    d_temb = nc.dram_tensor("temb", [128, HC, 2], F32, kind="ExternalInput")
    d_pw = nc.dram_tensor("pw", [128, 2, H], BF16, kind="ExternalInput")
    d_wuv = nc.dram_tensor("wuv", [nl, 128, HP, 2, WUV_W], FP8,
                           kind="ExternalInput")
    if OP_FP8:
        d_wout = nc.dram_tensor("wout", [nl, 128, EP, 2, H], FP8,
                                kind="ExternalInput")
    else:
        d_wout = nc.dram_tensor("wout", [nl, 128, EC, H], BF16,
                                kind="ExternalInput")
    d_upw = nc.dram_tensor("upw", [128, HC, PD], BF16, kind="ExternalInput")
    d_sperm = nc.dram_tensor("sperm", [128, 128], BF16, kind="ExternalInput")
    d_cq = nc.dram_tensor("cq", [128, TOK], BF16, kind="ExternalInput")
    d_sq = nc.dram_tensor("sq", [128, TOK], BF16, kind="ExternalInput")
    d_ck = nc.dram_tensor("ck", [128, TOK], BF16, kind="ExternalInput")
    d_sk = nc.dram_tensor("sk", [128, TOK], BF16, kind="ExternalInput")
    d_out = nc.dram_tensor("outt", [PD, TOK], F32, kind="ExternalOutput")

    from contextlib import ExitStack

    with tile.TileContext(nc) as tc, ExitStack() as ctx:
        pers = ctx.enter_context(tc.tile_pool(name="pers", bufs=1))
        wuvp = ctx.enter_context(tc.tile_pool(name="wuvp", bufs=3))
        woutp = ctx.enter_context(tc.tile_pool(name="woutp", bufs=2))
        rtmp = ctx.enter_context(tc.tile_pool(name="rtmp", bufs=1))
        hsqp = ctx.enter_context(tc.tile_pool(name="hsqp", bufs=2))
        utmp = ctx.enter_context(tc.tile_pool(name="utmp", bufs=3))
        attnp = ctx.enter_context(tc.tile_pool(name="attnp", bufs=3))
        statp = ctx.enter_context(tc.tile_pool(name="statp", bufs=4))
        rmsp = ctx.enter_context(tc.tile_pool(name="rmsp", bufs=1))
        rbp = ctx.enter_context(tc.tile_pool(name="rbp", bufs=2))

        psum = ctx.enter_context(tc.tile_pool(name="psum", bufs=1, space="PSUM"))

        # ---- persistent state + constants ----
        hTp = [pers.tile([128, 2, TOK], F32, name=f"hTp{p}", tag=f"hTp{p}")
               for p in range(HP)]
        # fp8 copy of raw h, stored as pair tiles for DoubleRow APs
        hf8 = [pers.tile([128, 2, TOK], FP8, name=f"hf8_{p}", tag=f"hf8_{p}")
               for p in range(HP)]
        uT = [pers.tile([128, TOK], BF16, name=f"uT{e}", tag=f"uT{e}")
              for e in range(EC)]
        # v natural per image: [tok-chunk pair, E]
        vn8 = [pers.tile([128, 2, E], FP8, name=f"vn8_{i}", tag=f"vn8_{i}")
               for i in range(2)]
        if OP_FP8:
            og8 = [pers.tile([128, 2, TOK], FP8, name=f"og8_{e}",
                             tag=f"og8_{e}") for e in range(EP)]
        else:
            ogT = [pers.tile([128, TOK], BF16, name=f"ogT{e}", tag=f"ogT{e}")
                   for e in range(EC)]
        # transposed attn per image [m-chunk, l]
        att2 = [pers.tile([128, 2, 256], FP8, name=f"att2_{i}",
                          tag=f"att2_{i}") for i in range(2)]
        qp = pers.tile([128, TOK], BF16)           # roped q (scaled)
        kp = pers.tile([128, TOK], BF16)           # roped k
        cq = pers.tile([128, TOK], BF16)
        sq = pers.tile([128, TOK], BF16)
        ck = pers.tile([128, TOK], BF16)
        sk = pers.tile([128, TOK], BF16)
        temb = pers.tile([128, HC, 2], F32)
        xpt = pers.tile([128, 2, TOK], BF16)
        pw = pers.tile([128, 2, H], BF16)
        upw = pers.tile([128, HC, PD], BF16)
        ones = pers.tile([128, 1], BF16)           # sumsq ones
        ones1 = pers.tile([1, 1], F32)
        sperm = pers.tile([128, 128], BF16)
        qsb = pers.tile([128, TOK], BF16)
        ksb = pers.tile([128, TOK], BF16)
        qsw = pers.tile([128, TOK], BF16)          # swapped-half copies
        ksw = pers.tile([128, TOK], BF16)
        ident = pers.tile([128, 128], BF16)
        identf = pers.tile([128, 128], F32)
        hfin = [pers.tile([128, TOK], BF16, name=f"hfin{j}", tag=f"hfin{j}")
                for j in range(HC)]

        nc.sync.dma_start(cq, d_cq.ap())
        nc.sync.dma_start(sq, d_sq.ap())
        nc.sync.dma_start(ck, d_ck.ap())
        nc.sync.dma_start(sk, d_sk.ap())
        nc.sync.dma_start(temb, d_temb.ap())
        nc.sync.dma_start(xpt, d_xpt.ap())
        nc.sync.dma_start(pw, d_pw.ap())
        nc.sync.dma_start(upw, d_upw.ap())
        nc.sync.dma_start(sperm, d_sperm.ap())
        nc.vector.memset(ones, 1.0)
        nc.vector.memset(ones1, 1.0)
        make_identity(nc, ident)
        make_identity(nc, identf)

        # ---- patchify: hT = patch_W.T @ xp.T + temb ----
        for j in range(HC):
            ps = psum.tile([128, TOK], F32, tag=f"p{1 + j % 2}")
            for c in range(2):
                nc.tensor.matmul(ps, pw[:, c, j * 128:(j + 1) * 128],
                                 xpt[:, c, :], start=(c == 0), stop=(c == 1))
            for i in range(2):
                nc.vector.tensor_scalar_add(
                    hTp[j // 2][:, j % 2, i * 256:(i + 1) * 256],
                    ps[:, i * 256:(i + 1) * 256],
                    temb[:, j, i:i + 1])
            nc.scalar.copy(hf8[j // 2][:, j % 2, :], hTp[j // 2][:, j % 2, :])

        def make_squares(name):
            hsq = hsqp.tile([128, HC, TOK], BF16, tag="hsq",
                            name=f"hsq_{name}")
            for jp in range(HP):
                nc.scalar.square(hsq[:, 2 * jp:2 * jp + 2, :],
                                 hTp[jp][:, :, :])
            return hsq

        hsq_next = make_squares("init")

        def rms_ss(hsq, name):
            """Sumsq over H via ones-matmul, evacuated to SBUF."""
            ss = psum.tile([1, TOK], F32, tag="p0", name=f"ss_{name}")
            for j in range(HC):
                nc.tensor.matmul(ss, ones, hsq[:, j, :],
                                 start=(j == 0), stop=(j == HC - 1))
            ssb = rmsp.tile([1, TOK], F32, tag="ssb", name=f"ssb_{name}")
            nc.scalar.copy(ssb, ss)
            return ssb

        def rms_newton(ssb, name, smul):
            """Newton rsqrt -> rt [128,4] (token-partition layout),
            scaled by 4*smul. The (1,512)<->(128,4) shuffles are done
            with tiny PE matmuls (K=1 scatter / M=1 gather)."""
            sst = psum.tile([128, 4], F32, tag="p0", name=f"sst_{name}")
            for t in range(4):
                nc.tensor.matmul(sst[:, t:t + 1], ssb[:, t * 128:(t + 1) * 128],
                                 ones1, start=True, stop=True)
            # m' = 16*mean(h^2) ~= 1.2 for this model; clamp to the seed's
            # convergence window; y = rsqrt(m')*4*smul = smul/rms.
            m = rmsp.tile([128, 4], F32, tag="m", name=f"m_{name}")
            nc.vector.tensor_scalar(m, sst, 16.0 / H, None, MUL)
            nc.vector.tensor_scalar(m, m, 0.15, 6.0,
                                    mybir.AluOpType.max,
                                    mybir.AluOpType.min)
            rt = rbp.tile([128, 4], F32, tag="rt", name=f"rt_{name}")
            t1 = rmsp.tile([128, 4], F32, tag="t1", name=f"t1_{name}")
            nc.vector.reciprocal(rt, m)
            nc.vector.tensor_scalar(rt, rt, 0.5, 0.5, MUL, ADD)
            # seed err <2% on the clamped window; 2 Newtons -> ~1e-6
            for _ in range(2):   # newton: y *= 1.5 - 0.5*m*y^2
                nc.vector.tensor_mul(t1, rt, rt)
                nc.vector.tensor_mul(t1, t1, m)
                nc.vector.tensor_scalar(t1, t1, -0.5, 1.5, MUL, ADD)
                nc.vector.tensor_mul(rt, rt, t1)
            nc.vector.tensor_scalar_mul(rt, rt, 4.0 * smul)
            return rt

        def rms_stats_b(rt, name):
            """Part 2: gather rt back to a (1,TOK) row and broadcast to a
            bf16 (128,TOK) tile for the (2x-rate) DVE multiplies."""
            row = psum.tile([1, TOK], F32, tag="p0", name=f"row_{name}")
            for t in range(4):
                nc.tensor.matmul(row[:, t * 128:(t + 1) * 128], rt[:, t:t + 1],
                                 identf, start=True, stop=True)
            yrow = rmsp.tile([1, TOK], BF16, tag="yrow", name=f"yrow_{name}")
            nc.scalar.copy(yrow, row)
            rb = rbp.tile([128, TOK], BF16, tag="rb", name=f"rb_{name}")
            nc.gpsimd.partition_broadcast(rb, yrow)
            return rb

        for lrep in range(nl * repeat):
            li = lrep % nl
            wuv = wuvp.tile([128, HP, 2, WUV_W], FP8, tag="wuv")
            nc.sync.dma_start(wuv, d_wuv.ap()[li])
            if OP_FP8:
                wout = woutp.tile([128, EP, 2, H], FP8, tag="wout")
            else:
                wout = woutp.tile([128, EC, H], BF16, tag="wout")
            nc.sync.dma_start(wout, d_wout.ap()[li])

            # ---- q/k col-tiles (weight-stationary, raw h); the swapped
            #      halves come from one permutation matmul each, evacuated
            #      to SBUF bf16 so rope runs fully in bf16 off SBUF.
            #      The rms sumsq / Newton stages weave between these so no
            #      engine queue-blocks on another. ----
            for t, sb in ((0, qsb), (1, ksb)):
                ct0 = E + t * 128
                ps = psum.tile([128, TOK], F32, tag=f"p{1 + t}")
                for jp in range(HP):
                    nc.tensor.matmul(ps, wuv[:, jp, :, ct0:ct0 + 128],
                                     hf8[jp][:, :, :], start=(jp == 0),
                                     stop=(jp == HP - 1), perf_mode=DR)
                nc.scalar.copy(sb, ps)

            ssb = rms_ss(hsq_next, f"l{lrep}")
            rt = rms_newton(ssb, f"l{lrep}", 1.0 / WSCALE)

            # ---- v natural (activation-stationary, raw h);
            #      silu(ps * rt) with per-partition (token) scale.
            #      3 psum banks per tk accumulate the jp pairs in
            #      parallel; the first half uses 6 banks so all 18
            #      matmuls issue before any silu (rt) is needed. ----
            VT = {0: ("p3", "p4", "p5"), 1: ("p6", "p7", "p1"),
                  2: ("p3", "p4", "p5"), 3: ("p6", "p7", "p1")}

            def v_tk(tk):
                vps = [psum.tile([128, 512], F32, tag=VT[tk][ns],
                                 name=f"vps{lrep}_{tk}_{ns}")
                       for ns in range(3)]
                for jp in range(HP):
                    for ns in range(3):
                        nc.tensor.matmul(
                            vps[ns],
                            hf8[jp][:, :, tk * 128:(tk + 1) * 128],
                            wuv[:, jp, :,
                                V0 + ns * 512:V0 + (ns + 1) * 512],
                            start=(jp == 0), stop=(jp == HP - 1),
                            perf_mode=DR)
                for ns in range(3):
                    nc.scalar.activation(
                        vn8[tk // 2][:, tk % 2, ns * 512:(ns + 1) * 512],
                        vps[ns], AF.Silu, scale=rt[:, tk:tk + 1])

            v_tk(0)
            v_tk(1)

            # swap-half matmuls + rb gather after the first v half: the
            # PE work above covers the ACT/DVE stats chain latency.
            qs_ps = psum.tile([128, TOK], F32, tag="p2")
            nc.tensor.matmul(qs_ps, sperm, qsb, start=True, stop=True)
            ks_ps = psum.tile([128, TOK], F32, tag="p0")
            nc.tensor.matmul(ks_ps, sperm, ksb, start=True, stop=True)
            nc.scalar.copy(qsw, qs_ps)
            nc.scalar.copy(ksw, ks_ps)

            # rope mix (no rb yet): mq = Q*cos + Qswap*sins -- all bf16
            mq = rtmp.tile([128, TOK], BF16, tag="mq")
            m2 = rtmp.tile([128, TOK], BF16, tag="m2")
            mk = rtmp.tile([128, TOK], BF16, tag="mk")
            m4 = rtmp.tile([128, TOK], BF16, tag="m4")
            nc.vector.tensor_mul(mq, qsb, cq)
            nc.vector.tensor_mul(m2, qsw, sq)
            nc.vector.tensor_add(mq, mq, m2)
            nc.vector.tensor_mul(mk, ksb, ck)
            nc.vector.tensor_mul(m4, ksw, sk)
            nc.vector.tensor_add(mk, mk, m4)

            rb = rms_stats_b(rt, f"l{lrep}")
            nc.vector.tensor_mul(qp, mq, rb)
            nc.vector.tensor_mul(kp, mk, rb)

            v_tk(2)

            # ---- scores + softmax (per image, per l-chunk); attn scaled
            #      x16 into fp8 range at the normalize step. Slotted
            #      between the second-half v GEMMs so exp/normalize clear
            #      ACT/DVE early and the transposes never stall. ----
            attn_sb = {}
            for i in range(2):
                for c in range(2):
                    sc = psum.tile([128, 256], F32,
                                   tag=("p2", "p1")[(i * 2 + c) % 2])
                    nc.tensor.matmul(sc, qp[:, i * 256 + c * 128:
                                            i * 256 + (c + 1) * 128],
                                     kp[:, i * 256:(i + 1) * 256],
                                     start=True, stop=True)
                    at = attnp.tile([128, 256], BF16, tag="attn")
                    sume = statp.tile([128, 1], F32, tag="sume")
                    nc.scalar.activation(at, sc, AF.Exp, scale=1.0,
                                         accum_out=sume)
                    rec = statp.tile([128, 1], F32, tag="rec")
                    nc.vector.reciprocal(rec, sume)
                    nc.vector.tensor_scalar(at, at, rec, ASCALE, MUL, MUL)
                    attn_sb[(i, c)] = at

            v_tk(3)

            # ---- transpose attn (PE) into fp8 pair tiles; right after
            #      scores so att2 is ready long before attn@v ----
            for i in range(2):
                for m in range(2):
                    aps = psum.tile([128, 256], BF16,
                                    tag=f"p{5 + (i * 2 + m) % 2}")
                    for c in range(2):
                        nc.tensor.transpose(
                            aps[:, c * 128:(c + 1) * 128],
                            attn_sb[(i, c)][:, m * 128:(m + 1) * 128],
                            ident)
                    nc.vector.tensor_copy(att2[i][:, m, :], aps)

            # ---- u col-tiles (weight-stationary, raw h) ----
            utags = ("p3", "p4", "p1", "p2")
            for ct in range(EC):
                ps = psum.tile([128, TOK], F32, tag=utags[ct % 4])
                for jp in range(HP):
                    nc.tensor.matmul(ps, wuv[:, jp, :, ct * 128:(ct + 1) * 128],
                                     hf8[jp][:, :, :], start=(jp == 0),
                                     stop=(jp == HP - 1), perf_mode=DR)
                ut = utmp.tile([128, TOK], BF16, tag="ut")
                nc.vector.tensor_mul(ut, ps, rb)
                nc.scalar.activation(uT[ct], ut, AF.Silu)

            # ---- oT = (attn @ v).T ; gate with uT into og. e-major, both
            #      images into one psum bank per e-chunk so the gating is
            #      one [128,512] DVE op per e-chunk; each og chunk then
            #      completes early for the contraction-outer
            #      out-projection below. ----
            for e in range(EC):
                ops = psum.tile([128, 512], F32, tag=("p7", "p0")[e % 2],
                                name=f"ops{lrep}_{e}")
                for i in range(2):
                    nc.tensor.matmul(ops[:, i * 256:(i + 1) * 256],
                                     vn8[i][:, :, e * 128:(e + 1) * 128],
                                     att2[i][:, :, :],
                                     start=True, stop=True, perf_mode=DR)
                dst = og8[e // 2][:, e % 2, :] if OP_FP8 else ogT[e]
                nc.vector.tensor_mul(dst, uT[e], ops)

            # ---- out-projection, contraction-outer: all 6 dps banks
            #      accumulate in parallel, each e-round firing as soon as
            #      its og chunk is gated. The final round is per-hp
            #      chained into residual (x RES_SCALE) / squares / fp8
            #      copies so banks and h state free incrementally. ----
            hsq_next = hsqp.tile([128, HC, TOK], BF16, tag="hsq",
                                 name=f"hsq_l{lrep}")
            dpss = [psum.tile([128, TOK], F32, tag=f"p{1 + hp}",
                              name=f"dps{lrep}_{hp}") for hp in range(HC)]

            def op_mm(e_or_ep, hp, start, stop):
                if OP_FP8:
                    nc.tensor.matmul(dpss[hp],
                                     wout[:, e_or_ep, :,
                                          hp * 128:(hp + 1) * 128],
                                     og8[e_or_ep][:, :, :], start=start,
                                     stop=stop, perf_mode=DR)
                else:
                    nc.tensor.matmul(dpss[hp],
                                     wout[:, e_or_ep,
                                          hp * 128:(hp + 1) * 128],
                                     ogT[e_or_ep], start=start, stop=stop)

            n_rounds = EP if OP_FP8 else EC
            for r in range(n_rounds - 1):
                for hp in range(HC):
                    op_mm(r, hp, r == 0, False)
            for hp in range(HC):
                op_mm(n_rounds - 1, hp, False, True)
                nc.vector.scalar_tensor_tensor(hTp[hp // 2][:, hp % 2, :],
                                               dpss[hp], RES_SCALE,
                                               hTp[hp // 2][:, hp % 2, :],
                                               MUL, ADD)
                if hp % 2 == 1:
                    jp = hp // 2
                    # fp8 copy first (unblocks next layer's qk GEMMs),
                    # then the square (feeds its sumsq); both on ACT so
                    # the DVE drains to Newton sooner.
                    nc.scalar.copy(hf8[jp][:, :, :], hTp[jp][:, :, :])
                    nc.scalar.square(hsq_next[:, 2 * jp:2 * jp + 2, :],
                                     hTp[jp][:, :, :])

        # ---- final norm + unpatch (fnorm_w folded into upw on host) ----
        ssb = rms_ss(hsq_next, "fin")
        rt = rms_newton(ssb, "fin", 1.0)
        rb = rms_stats_b(rt, "fin")
        for j in range(HC):
            nc.vector.tensor_mul(hfin[j], hTp[j // 2][:, j % 2, :], rb)
        for mchunk, msz in ((0, 128), (1, 64)):
            ps = psum.tile([128, TOK], F32, tag=f"p{5 + mchunk}")
            for j in range(HC):
                nc.tensor.matmul(ps[:msz, :],
                                 upw[:, j, mchunk * 128:mchunk * 128 + msz],
                                 hfin[j], start=(j == 0),
                                 stop=(j == HC - 1))
            osb = rtmp.tile([128, TOK], F32, tag="osb")
            nc.vector.tensor_copy(osb[:msz, :], ps[:msz, :])
            nc.sync.dma_start(d_out.ap()[mchunk * 128:mchunk * 128 + msz, :],
                              osb[:msz, :])

    nc.compile()
    return nc


_BUILD_CACHE = {}


def _get_nc(nl=NL, repeat=1):
    key = (nl, repeat)
    if key not in _BUILD_CACHE:
        _BUILD_CACHE[key] = _build(nl, repeat)
    return _BUILD_CACHE[key]


def _rope_tables():
    pos = np.arange(L)

    def sinemb(p, dim=64, base=1000.0):
        half = dim // 2
        freqs = np.exp(np.arange(half, dtype=np.float32)
                       * np.float32(-np.log(base) / (half - 1)))
        ang = p[:, None].astype(np.float32) * freqs[None, :]
        return np.concatenate([np.sin(ang), np.cos(ang)], axis=-1)

    w = IMG // P
    pe = np.concatenate([sinemb(pos // w), sinemb(pos % w)],
                        axis=-1).astype(np.float32)      # (256, 128)
    sinv = pe[:, :64].T                                  # (64, 256)
    cosv = pe[:, 64:].T
    COS = np.concatenate([cosv, cosv], axis=0)           # (128, 256)
    SINS = np.concatenate([-sinv, sinv], axis=0)
    COS2 = np.tile(COS, (1, 2))                          # (128, 512)
    SINS2 = np.tile(SINS, (1, 2))
    scale = np.float32(KD ** -0.5)
    return (np.ascontiguousarray(COS2 * scale).astype(BF),
            np.ascontiguousarray(SINS2 * scale).astype(BF),
            np.ascontiguousarray(COS2).astype(BF),
            np.ascontiguousarray(SINS2).astype(BF))


def _q8(x):
    return np.clip(x * WSCALE, -240.0, 240.0).astype(F8)


def _prep_weights(patch_W, t_emb, Wuv, Wout, gnorm, fnorm_w, unpatch_W, nl=NL):
    Wg = Wuv[:nl] * gnorm[:nl, :, None]                  # fold gnorm
    u = Wg[:, :, :E]
    q = Wg[:, :, 2 * E:2 * E + KD]
    k = Wg[:, :, 2 * E + KD:]
    v = Wg[:, :, E:2 * E]
    wuvp = np.concatenate([u, q, k, v], axis=2)          # (nl, 768, 3328)
    # fp8 x64, laid out [nl, 128, HP, 2, W] so [:, jp, :, cols] is a
    # DoubleRow [128, 2, ncols] k-pair AP (h = (2*jp+i)*128 + p).
    wuv_h = np.ascontiguousarray(
        _q8(wuvp).reshape(nl, HP, 2, 128, WUV_W).transpose(0, 3, 1, 2, 4))
    if OP_FP8:
        wout_h = np.ascontiguousarray(
            _q8(Wout[:nl]).reshape(nl, EP, 2, 128, H).transpose(0, 3, 1, 2, 4))
    else:
        wout_h = np.ascontiguousarray(
            Wout[:nl].reshape(nl, EC, 128, H).transpose(0, 2, 1, 3)).astype(BF)
    pw_pad = np.zeros((256, H), np.float32)
    pw_pad[:PD] = patch_W
    pw_h = np.ascontiguousarray(
        pw_pad.reshape(2, 128, H).transpose(1, 0, 2)).astype(BF)
    upw = fnorm_w[:, None] * unpatch_W                   # fold fnorm
    upw_h = np.ascontiguousarray(
        upw.reshape(HC, 128, PD).transpose(1, 0, 2)).astype(BF)
    return wuv_h, wout_h, pw_h, upw_h


def _patchify(xc):
    """(2,3,128,128) -> (512, 192) token-major patches."""
    g = IMG // P
    xp = xc.reshape(2, 3, g, P, g, P).transpose(0, 2, 4, 3, 5, 1)
    return np.ascontiguousarray(xp.reshape(2 * L, PD))


def _unpatchify(oT):
    """(192, 512) -> (2, 3, 128, 128)."""
    g = IMG // P
    out = np.empty((2, 3, IMG, IMG), np.float32)
    for i in range(2):
        h = oT[:, i * L:(i + 1) * L].T                   # (256, 192)
        out[i] = (h.reshape(g, g, P, P, 3)
                  .transpose(4, 0, 2, 1, 3).reshape(3, IMG, IMG))
    return out


def make_in_maps(x, t_idx, patch_W, t_emb, Wuv, Wout, gnorm, fnorm_w,
                 unpatch_W, nl=NL):
    x = np.asarray(x, np.float32)
    t_idx = np.asarray(t_idx).astype(np.int64)
    patch_W = np.asarray(patch_W, np.float32)
    t_emb = np.asarray(t_emb, np.float32)
    Wuv = np.asarray(Wuv, np.float32)
    Wout = np.asarray(Wout, np.float32)
    gnorm = np.asarray(gnorm, np.float32)
    fnorm_w = np.asarray(fnorm_w, np.float32)
    unpatch_W = np.asarray(unpatch_W, np.float32)

    wuv_h, wout_h, pw_h, upw_h = _prep_weights(
        patch_W, t_emb, Wuv, Wout, gnorm, fnorm_w, unpatch_W, nl)
    cqt, sqt, ckt, skt = _rope_tables()
    sperm = np.ascontiguousarray(np.roll(np.eye(128, dtype=np.float32),
                                         64, axis=0)).astype(BF)

    in_maps = []
    for c in range(NCORES):
        xc = x[2 * c:2 * c + 2]
        xp = _patchify(xc)                               # (512, 192)
        xpad = np.zeros((TOK, 256), np.float32)
        xpad[:, :PD] = xp
        xpt = np.ascontiguousarray(
            xpad.T.reshape(2, 128, TOK).transpose(1, 0, 2)).astype(BF)
        te = t_emb[t_idx[2 * c:2 * c + 2, 0]]            # (2, 768)
        tembT = np.ascontiguousarray(
            te.T.reshape(HC, 128, 2).transpose(1, 0, 2)).astype(np.float32)
        in_maps.append({
            "xpt": xpt, "temb": tembT, "pw": pw_h, "wuv": wuv_h,
            "wout": wout_h, "upw": upw_h, "cq": cqt, "sq": sqt,
            "ck": ckt, "sk": skt, "sperm": sperm,
        })
    return in_maps


def kernel(**inputs):
    from concourse.bass_utils import run_bass_kernel_spmd

    nc = _get_nc()
    in_maps = make_in_maps(**inputs)
    res = run_bass_kernel_spmd(nc, in_maps, core_ids=list(range(NCORES)))
    out = np.empty((B, 3, IMG, IMG), np.float32)
    for c in range(NCORES):
        out[2 * c:2 * c + 2] = _unpatchify(res.results[c]["outt"])
    return out
